# revision 1
# baseline (speedup 1.0000x reference)
"""Trainium2 Bass kernel for the cross-attention transformer block.

Strategy (8 NeuronCores, data-parallel over batch B=8, one batch item per core):
  - All activations live FEATURE-MAJOR on chip ([feature, token]) so every
    activation x weight matmul contracts over the partition dim with zero
    on-chip transposes.  The host pre-transposes x/y per batch item and
    re-transposes the output.
  - LayerNorm stats (mean / mean-of-squares over features) are computed with
    ones-vector matmuls on the PE (partition-dim reduction), broadcast back to
    [128, N] via K=1 outer-product matmuls, and applied with two DVE
    tensor-tensor passes.
  - All big matmuls run in bf16 (1 cycle/row on PE vs 2 for fp32) with fp32
    PSUM accumulation.
  - Attention is computed key-major: scores^T = K^T(d,m)·Q(d,n) with 4 heads
    row-packed into the 128x128 array (K=32 each); softmax skips the max
    subtraction (scores are ~N(0,1), |s|<6 always for these shapes); exp runs
    on ACT straight out of PSUM two heads at a time (FD=1024).  V carries an
    appended ones column per head ([m, 33] stationary), so the AV matmul
    produces both o_unnorm (rows 0..31) and the softmax denominator Z (row 32)
    in one pass; Z is broadcast back across each head's partitions with a tiny
    indicator matmul and applied as 1/Z on DVE.
  - FFN weights (W2/W3, 16 MB bf16) are streamed from HBM.
"""

import sys

for _p in ("/opt/trn_rl_repo", "/root/.axon_site/_ro/trn_rl_repo"):
    if _p not in sys.path:
        sys.path.append(_p)

import numpy as np
import ml_dtypes

import concourse.bacc as bacc
import concourse.mybir as mybir
from concourse.tile import TileContext
from concourse import bass_utils

F32 = mybir.dt.float32
BF16 = mybir.dt.bfloat16
AF = mybir.ActivationFunctionType
OP = mybir.AluOpType

P = 128
B, N, C, H, D, W = 8, 1024, 1024, 16, 32, 4
HD = H * D            # 512
DA = 2 * D            # 64: V + ones column, padded for aligned LDWEIGHTS
F = W * C             # 4096
KT = C // P           # 8 feature k-tiles
NB = N // 512         # 2 column blocks of 512 tokens
EPS = 1e-5
NCORES = 8

_BUILD_CACHE = {}
_LAST_IN_MAPS = None


def _emit_ln(nc, pools, psum, src_f32, src_bf, out_bf, out_f32, gb, name):
    """Feature-major layernorm over the partition (feature) axis."""
    p_sq, p_stats, p_small = pools["sq"], pools["stats"], pools["small"]
    ones_col, ones_row, eps_tile = (
        pools["ones_col"], pools["ones_row"], pools["eps"])
    ones_colf = pools["ones_colf"]

    # squares for E[x^2] (ACT, fp32 in -> bf16 out)
    sq = []
    for k in range(KT):
        x2 = p_sq.tile([P, N], BF16, name=f"{name}_sq{k}", tag="sq")
        nc.scalar.activation(out=x2, in_=src_f32[k], func=AF.Square)
        sq.append(x2)

    # S1 = sum_c x, S2 = sum_c x^2 : ones-vector matmuls, accumulated over k
    s1_sb = p_small.tile([1, N], F32, name=f"{name}_s1row", tag="row1024", bufs=2)
    s2_sb = p_small.tile([1, N], F32, name=f"{name}_s2row", tag="row1024", bufs=2)
    s1_rhs = src_bf if src_bf is not None else src_f32
    s1_ones = ones_col if src_bf is not None else ones_colf
    for dst_sb, rhs_tiles, ones_t, idx in ((s1_sb, s1_rhs, s1_ones, 0),
                                           (s2_sb, sq, ones_col, 1)):
        for nb in range(NB):
            s_ps = psum.tile([1, 512], F32, name=f"{name}_sps{idx}{nb}", tag="ps")
            for k in range(KT):
                nc.tensor.matmul(
                    s_ps[0:1, :], ones_t[:, 0:1],
                    rhs_tiles[k][:, nb * 512:(nb + 1) * 512],
                    start=(k == 0), stop=(k == KT - 1))
            nc.scalar.copy(out=dst_sb[0:1, nb * 512:(nb + 1) * 512],
                           in_=s_ps[0:1, :])

    # broadcast S1, S2 down all 128 partitions via K=1 fp32 matmuls
    mu_b = p_stats.tile([P, N], F32, name=f"{name}_mu", tag="st")
    ex2_b = p_stats.tile([P, N], F32, name=f"{name}_ex2", tag="st")
    for src_sb, dst, idx in ((s1_sb, mu_b, 0), (s2_sb, ex2_b, 1)):
        for nb in range(NB):
            bc_ps = psum.tile([P, 512], F32, name=f"{name}_bc{idx}{nb}", tag="ps")
            nc.tensor.matmul(bc_ps, ones_row[0:1, :],
                             src_sb[0:1, nb * 512:(nb + 1) * 512],
                             start=True, stop=True)
            nc.vector.tensor_scalar_mul(
                out=dst[:, nb * 512:(nb + 1) * 512], in0=bc_ps, scalar1=1.0 / C)

    # rstd = 1/sqrt(E[x^2] - mu^2 + eps); mr = mu * rstd
    mu2_b = p_stats.tile([P, N], F32, name=f"{name}_mu2", tag="st")
    nc.scalar.activation(out=mu2_b, in_=mu_b, func=AF.Square)
    var_b = mu2_b  # reuse
    nc.vector.tensor_tensor(out=var_b, in0=ex2_b, in1=mu2_b, op=OP.subtract)
    sd_b = ex2_b  # reuse
    nc.scalar.activation(out=sd_b, in_=var_b, func=AF.Sqrt, bias=eps_tile[:, 0:1])
    rstd_b = p_stats.tile([P, N], F32, name=f"{name}_rstd", tag="st")
    nc.vector.reciprocal_approx_fast(out=rstd_b, in_=sd_b)
    mr_b = p_stats.tile([P, N], F32, name=f"{name}_mr", tag="st")
    nc.vector.tensor_mul(out=mr_b, in0=mu_b, in1=rstd_b)

    # apply: xn = x*rstd - mr  (then optional gamma/beta per-feature)
    for k in range(KT):
        t0 = p_stats.tile([P, N], F32, name=f"{name}_t{k}", tag="st")
        nc.vector.tensor_mul(out=t0, in0=src_f32[k], in1=rstd_b)
        if out_f32 is not None:
            xn = pools["res"].tile([P, N], F32, name=f"{name}_f{k}", tag="res")
            nc.vector.tensor_tensor(out=xn, in0=t0, in1=mr_b, op=OP.subtract)
            if gb is not None:
                nc.vector.tensor_scalar(
                    out=xn, in0=xn, scalar1=gb[0][k], scalar2=gb[1][k],
                    op0=OP.mult, op1=OP.add)
            out_f32.append(xn)
            xb = pools["bfa"].tile([P, N], BF16, name=f"{name}_b{k}", tag="bfa")
            nc.vector.tensor_copy(out=xb, in_=xn)
            out_bf.append(xb)
        else:
            xb = pools["bfa"].tile([P, N], BF16, name=f"{name}_b{k}", tag="bfa")
            if gb is not None:
                t1 = p_stats.tile([P, N], F32, name=f"{name}_u{k}", tag="st")
                nc.vector.tensor_tensor(out=t1, in0=t0, in1=mr_b, op=OP.subtract)
                nc.vector.tensor_scalar(
                    out=xb, in0=t1, scalar1=gb[0][k], scalar2=gb[1][k],
                    op0=OP.mult, op1=OP.add)
            else:
                nc.vector.tensor_tensor(out=xb, in0=t0, in1=mr_b, op=OP.subtract)
            out_bf.append(xb)


def _build(flags):
    """Build + finalize the Bass program. flags = (g1be1, g2be2, g3be3, b1, b2, b3)"""
    f_g1, f_g2, f_g3, f_b1, f_b2, f_b3 = flags
    nc = bacc.Bacc("TRN2", target_bir_lowering=False)

    xT = nc.dram_tensor("xT", [C, N], F32, kind="ExternalInput")
    yT = nc.dram_tensor("yT", [C, N], F32, kind="ExternalInput")
    xTb = nc.dram_tensor("xTb", [C, N], BF16, kind="ExternalInput")
    yTb = nc.dram_tensor("yTb", [C, N], BF16, kind="ExternalInput")
    wq = nc.dram_tensor("wq", [C, HD], BF16, kind="ExternalInput")
    wk = nc.dram_tensor("wk", [C, HD], BF16, kind="ExternalInput")
    wv = nc.dram_tensor("wv", [C, HD], BF16, kind="ExternalInput")
    w1 = nc.dram_tensor("w1", [HD, C], BF16, kind="ExternalInput")
    w2 = nc.dram_tensor("w2", [C, F], BF16, kind="ExternalInput")
    w3 = nc.dram_tensor("w3", [F, C], BF16, kind="ExternalInput")
    vecs = {}
    if f_g1:
        vecs["g1"] = nc.dram_tensor("g1", [C, 1], F32, kind="ExternalInput")
        vecs["be1"] = nc.dram_tensor("be1", [C, 1], F32, kind="ExternalInput")
    if f_g2:
        vecs["g2"] = nc.dram_tensor("g2", [C, 1], F32, kind="ExternalInput")
        vecs["be2"] = nc.dram_tensor("be2", [C, 1], F32, kind="ExternalInput")
    if f_g3:
        vecs["g3"] = nc.dram_tensor("g3", [C, 1], F32, kind="ExternalInput")
        vecs["be3"] = nc.dram_tensor("be3", [C, 1], F32, kind="ExternalInput")
    if f_b1:
        vecs["b1"] = nc.dram_tensor("b1", [C, 1], F32, kind="ExternalInput")
    if f_b2:
        vecs["b2"] = nc.dram_tensor("b2", [F, 1], F32, kind="ExternalInput")
    if f_b3:
        vecs["b3"] = nc.dram_tensor("b3", [C, 1], F32, kind="ExternalInput")
    OT = nc.dram_tensor("OT", [C, N], F32, kind="ExternalOutput")

    with TileContext(nc) as tc:
        with (
            tc.tile_pool(name="p_small", bufs=4) as p_small,
            tc.tile_pool(name="p_stats", bufs=5) as p_stats,
            tc.tile_pool(name="p_sq", bufs=4) as p_sq,
            tc.tile_pool(name="p_bfa", bufs=16) as p_bfa,
            tc.tile_pool(name="p_res", bufs=9) as p_res,
        ):
            ones_col = p_small.tile([P, 1], BF16, name="ones_col", tag="p1")
            nc.vector.memset(ones_col, 1.0)
            ones_row = p_small.tile([1, P], F32, name="ones_row", tag="p1")
            nc.vector.memset(ones_row, 1.0)
            eps_tile = p_small.tile([P, 1], F32, name="eps_tile", tag="p1")
            nc.vector.memset(eps_tile, EPS)
            ones_colf = p_small.tile([P, 1], F32, name="ones_colf",
                                     tag="p1f", bufs=1)
            nc.vector.memset(ones_colf, 1.0)
            # e4[k, p] = 1 iff k == 32*(p//32): broadcasts a Z row stored at
            # partition 32j to the 32 output partitions of head j
            e4 = p_small.tile([P, P], F32, name="e4", tag="e4", bufs=1)
            nc.vector.memset(e4, 0.0)
            for j in range(4):
                nc.vector.memset(e4[32 * j:32 * j + 1, 32 * j:32 * (j + 1)], 1.0)

            vec_tiles = {}
            for vn, dram in vecs.items():
                nparts = dram.shape[0] // P
                tiles = []
                for k in range(nparts):
                    t = p_small.tile([P, 1], F32, name=f"{vn}_{k}", tag="p1",
                                     bufs=nparts + 1)
                    nc.sync.dma_start(out=t, in_=dram[k * P:(k + 1) * P, 0:1])
                    tiles.append(t)
                vec_tiles[vn] = tiles

            pools = {
                "sq": p_sq, "stats": p_stats, "small": p_small,
                "bfa": p_bfa, "res": p_res, "ones_colf": ones_colf,
                "ones_col": ones_col, "ones_row": ones_row, "eps": eps_tile,
            }

            xnb, ynb, xn_f32 = [], [], []
            qT, kTt, vtok, oT = [], [], [], []

            with (
                tc.tile_pool(name="p_qk", bufs=13) as p_qk,
                tc.tile_pool(name="p_w1", bufs=5) as p_w1,
            ):
                with tc.tile_pool(name="p_spare", bufs=1) as _sp:
                    with (
                        tc.tile_pool(name="psA", bufs=4, space="PSUM") as psA,
                        tc.tile_pool(name="p_wqkv", bufs=16) as p_wqkv,
                    ):
                        # ---------- Phase A: load + layernorm x and y ----------
                        with tc.tile_pool(name="p_in", bufs=8) as p_in:
                            for nm, dram, dramb, obf, of32, fl, gk in (
                                    ("lnx", xT, xTb, xnb, xn_f32, f_g1, "1"),
                                    ("lny", yT, yTb, ynb, None, f_g2, "2")):
                                src_f, src_b = [], []
                                for k in range(KT):
                                    tf = p_in.tile([P, N], F32,
                                                   name=f"{nm}_in{k}", tag="inf")
                                    nc.sync.dma_start(
                                        out=tf, in_=dram[k * P:(k + 1) * P, :])
                                    src_f.append(tf)
                                    tb = p_sq.tile([P, N], BF16,
                                                   name=f"{nm}_inb{k}", tag="sq")
                                    nc.gpsimd.dma_start(
                                        out=tb, in_=dramb[k * P:(k + 1) * P, :])
                                    src_b.append(tb)
                                gb = None
                                if fl:
                                    gb = (vec_tiles["g" + gk],
                                          vec_tiles["be" + gk])
                                _emit_ln(nc, pools, psA, src_f, src_b, obf,
                                         of32, gb, nm)

                        # projection weights (issued after the LN input DMAs
                        # so x/y loads own the queue early)
                        wq_sb, wk_sb = [], []
                        for wn, dram, lst in (("wq", wq, wq_sb),
                                              ("wk", wk, wk_sb)):
                            for k in range(KT):
                                t = p_wqkv.tile([P, HD], BF16, name=f"{wn}s{k}",
                                                tag="wqkv")
                                nc.sync.dma_start(out=t,
                                                  in_=dram[k * P:(k + 1) * P, :])
                                lst.append(t)
                        w1_sb = []
                        for g in range(4):
                            t = p_w1.tile([P, C], BF16, name=f"w1s{g}", tag="w1")
                            nc.sync.dma_start(out=t, in_=w1[g * P:(g + 1) * P, :])
                            w1_sb.append(t)

                        # ---------- Phase B: QKV projections ----------
                        for g in range(4):
                            for dst, nm in ((qT, "q"), (kTt, "k")):
                                dst.append(p_qk.tile([P, N], BF16,
                                                     name=f"{nm}T{g}", tag="qk"))
                        for g in range(4):
                            for nb in range(NB):
                                for dst, src_act, wsb, nm in (
                                        (qT, xnb, wq_sb, "q"),
                                        (kTt, ynb, wk_sb, "k")):
                                    pp = psA.tile([P, 512], F32,
                                                  name=f"{nm}ps{g}{nb}", tag="ps")
                                    for k in range(KT):
                                        nc.tensor.matmul(
                                            pp, wsb[k][:, g * P:(g + 1) * P],
                                            src_act[k][:, nb * 512:(nb + 1) * 512],
                                            start=(k == 0), stop=(k == KT - 1))
                                    nc.vector.tensor_copy(
                                        out=dst[g][:, nb * 512:(nb + 1) * 512],
                                        in_=pp)

                    # ---------- Phase C: attention ----------
                    with (
                        tc.tile_pool(name="ps2", bufs=2, space="PSUM") as ps2,
                        tc.tile_pool(name="psC", bufs=4, space="PSUM") as psC,
                        tc.tile_pool(name="p_e", bufs=4) as p_e,
                        tc.tile_pool(name="p_zi", bufs=4) as p_zi,
                        tc.tile_pool(name="p_v", bufs=8) as p_v,
                        tc.tile_pool(name="p_wv", bufs=8) as p_wv,
                    ):
                        wv_sb = []
                        for k in range(KT):
                            t = p_wv.tile([P, HD], BF16, name=f"wvs{k}",
                                          tag="wv")
                            nc.sync.dma_start(out=t, in_=wv[k * P:(k + 1) * P, :])
                            wv_sb.append(t)
                        # v token-major, one ones-column per head (padded to 64)
                        for mt in range(KT):
                            vp = ps2.tile([P, HD], F32, name=f"vps{mt}",
                                          tag="ps2")
                            for k in range(KT):
                                nc.tensor.matmul(
                                    vp, ynb[k][:, mt * P:(mt + 1) * P], wv_sb[k],
                                    start=(k == 0), stop=(k == KT - 1))
                            vt = p_v.tile([P, H * DA], BF16, name=f"v{mt}",
                                          tag="v")
                            nc.vector.memset(vt, 1.0)
                            vt3 = vt.rearrange("p (h w) -> p h w", w=DA)
                            nc.vector.tensor_copy(
                                out=vt3[:, :, 0:D],
                                in_=vp.rearrange("p (h w) -> p h w", w=D))
                            vtok.append(vt)
                        for g in range(4):
                            oT.append(p_qk.tile([P, N], BF16,
                                                name=f"oT{g}", tag="qk"))
                        for g in range(4):
                            for nb in range(NB):
                                ns = slice(nb * 512, (nb + 1) * 512)
                                aug = [psC.tile([DA, 512], F32,
                                                name=f"aps{g}{nb}{j}", tag="psC")
                                       for j in range(4)]
                                for mt in range(KT):
                                    e_sb = []
                                    for pr in range(2):
                                        s_ps = ps2.tile(
                                            [P, 1024], F32,
                                            name=f"sps{g}{nb}{mt}{pr}", tag="ps2")
                                        for jj in range(2):
                                            j = 2 * pr + jj
                                            nc.tensor.matmul(
                                                s_ps[:, jj * 512:(jj + 1) * 512],
                                                kTt[g][32 * j:32 * (j + 1),
                                                       mt * P:(mt + 1) * P],
                                                qT[g][32 * j:32 * (j + 1), ns],
                                                start=True, stop=True,
                                                tile_position=(32 * j, 0))
                                        et = p_e.tile([P, 1024], BF16,
                                                      name=f"e{g}{nb}{mt}{pr}",
                                                      tag="e")
                                        nc.scalar.activation(
                                            out=et, in_=s_ps, func=AF.Exp)
                                        e_sb.append(et)
                                    for j in range(4):
                                        h = 4 * g + j
                                        erhs = e_sb[j // 2][:, (j % 2) * 512:
                                                            (j % 2 + 1) * 512]
                                        nc.tensor.matmul(
                                            aug[j], vtok[mt][:, h * DA:(h + 1) * DA],
                                            erhs,
                                            start=(mt == 0), stop=(mt == KT - 1))
                                # drain aug PSUM fast: unnormalized o and Z
                                # rows to SBUF, then normalize off the
                                # accumulation critical path
                                zrows = p_zi.tile([P, 512], F32,
                                                  name=f"zr{g}{nb}", tag="zi")
                                nc.vector.memset(zrows, 0.0)
                                o_un = p_zi.tile([P, 512], BF16,
                                                 name=f"ou{g}{nb}", tag="ou",
                                                 bufs=2)
                                for j in range(4):
                                    nc.vector.tensor_copy(
                                        out=zrows[32 * j:32 * j + 1, :],
                                        in_=aug[j][D:D + 1, :])
                                    nc.vector.tensor_copy(
                                        out=o_un[32 * j:32 * (j + 1), :],
                                        in_=aug[j][0:D, :])
                                zb_ps = ps2.tile([P, 512], F32,
                                                 name=f"zb{g}{nb}", tag="ps2")
                                nc.tensor.matmul(zb_ps, e4, zrows,
                                                 start=True, stop=True)
                                zsb = p_zi.tile([P, 512], F32,
                                                name=f"zs{g}{nb}", tag="zi")
                                nc.vector.tensor_copy(out=zsb, in_=zb_ps)
                                zinv = p_zi.tile([P, 512], F32,
                                                 name=f"zi{g}{nb}", tag="zi")
                                nc.vector.reciprocal_approx_fast(
                                    out=zinv, in_=zsb)
                                nc.vector.tensor_tensor(
                                    out=oT[g][:, ns], in0=o_un, in1=zinv,
                                    op=OP.mult)

                # ---------- Phase D: out1 = xn + o @ W1 (+b1) ----------
                out1 = []
                for ct in range(KT):
                    out1.append(p_res.tile([P, N], F32, name=f"out1_{ct}",
                                           tag="res"))
                with tc.tile_pool(name="psD", bufs=4, space="PSUM") as psD:
                    for ct in range(KT):
                        for nb in range(NB):
                            ns = slice(nb * 512, (nb + 1) * 512)
                            u_ps = psD.tile([P, 512], F32, name=f"ups{ct}{nb}",
                                            tag="ps")
                            for g in range(4):
                                nc.tensor.matmul(
                                    u_ps, w1_sb[g][:, ct * P:(ct + 1) * P],
                                    oT[g][:, ns],
                                    start=(g == 0), stop=(g == 3))
                            b1s = vec_tiles["b1"][ct] if f_b1 else 0.0
                            nc.vector.scalar_tensor_tensor(
                                out=out1[ct][:, ns], in0=u_ps, scalar=b1s,
                                in1=xn_f32[ct][:, ns], op0=OP.add, op1=OP.add)

            # ---------- Phase E: LN3 ----------
            with tc.tile_pool(name="psE", bufs=4, space="PSUM") as psE:
                ln3 = []
                gb3 = (vec_tiles["g3"], vec_tiles["be3"]) if f_g3 else None
                _emit_ln(nc, pools, psE, out1, None, ln3, None, gb3, "ln3")

                # ---------- Phase F: FFN ----------
                with (
                    tc.tile_pool(name="p_h1g", bufs=33) as p_h1g,
                    tc.tile_pool(name="p_w2", bufs=16) as p_w2,
                    tc.tile_pool(name="p_w3", bufs=3) as p_w3,
                    tc.tile_pool(name="p_fin", bufs=4) as p_fin,
                    tc.tile_pool(name="psF", bufs=4, space="PSUM") as psF,
                ):
                    for nb in range(NB):
                        ns = slice(nb * 512, (nb + 1) * 512)
                        h1g = []
                        for fq in range(8):
                            w2t = []
                            for ct in range(KT):
                                t = p_w2.tile([P, 512], BF16,
                                              name=f"w2t{nb}{fq}{ct}", tag="w2")
                                nc.sync.dma_start(
                                    out=t, in_=w2[ct * P:(ct + 1) * P,
                                                 fq * 512:(fq + 1) * 512])
                                w2t.append(t)
                            for fi in range(4):
                                ft = fq * 4 + fi
                                h_ps = psE.tile([P, 512], F32,
                                                name=f"hps{nb}{ft}", tag="ps")
                                for ct in range(KT):
                                    nc.tensor.matmul(
                                        h_ps, w2t[ct][:, fi * P:(fi + 1) * P],
                                        ln3[ct][:, ns],
                                        start=(ct == 0), stop=(ct == KT - 1))
                                hg = p_h1g.tile([P, 512], BF16,
                                                name=f"h1g{nb}{ft}", tag="h1g")
                                b2s = vec_tiles["b2"][ft] if f_b2 else 0.0
                                nc.scalar.activation(out=hg, in_=h_ps,
                                                     func=AF.Gelu, bias=b2s)
                                h1g.append(hg)
                        h2_ps = []
                        for ct in range(KT):
                            pool_f = psE if ct < 4 else psF
                            h2_ps.append(pool_f.tile(
                                [P, 512], F32, name=f"h2ps{nb}{ct}",
                                tag="ps" if ct < 4 else "psf"))
                        for ft in range(F // P):
                            w3t = p_w3.tile([P, C], BF16, name=f"w3t{nb}{ft}",
                                            tag="w3")
                            nc.sync.dma_start(out=w3t,
                                              in_=w3[ft * P:(ft + 1) * P, :])
                            for ct in range(KT):
                                nc.tensor.matmul(
                                    h2_ps[ct], w3t[:, ct * P:(ct + 1) * P],
                                    h1g[ft],
                                    start=(ft == 0), stop=(ft == F // P - 1))
                        for ct in range(KT):
                            fin = p_fin.tile([P, 512], F32, name=f"fin{nb}{ct}",
                                             tag="fin")
                            b3s = vec_tiles["b3"][ct] if f_b3 else 0.0
                            nc.vector.scalar_tensor_tensor(
                                out=fin, in0=h2_ps[ct], scalar=b3s,
                                in1=out1[ct][:, ns], op0=OP.add, op1=OP.add)
                            nc.sync.dma_start(out=OT[ct * P:(ct + 1) * P, ns],
                                              in_=fin)

    nc.finalize()
    return nc


def _nontrivial(v, val):
    return not np.allclose(np.asarray(v), val, rtol=0.0, atol=0.0)


def kernel(x, y, Wq, Wk, Wv, W1, b1, g1, be1, g2, be2, g3, be3, W2, b2, W3, b3):
    x = np.asarray(x, np.float32)
    y = np.asarray(y, np.float32)
    bf = ml_dtypes.bfloat16

    f_g1 = _nontrivial(g1, 1.0) or _nontrivial(be1, 0.0)
    f_g2 = _nontrivial(g2, 1.0) or _nontrivial(be2, 0.0)
    f_g3 = _nontrivial(g3, 1.0) or _nontrivial(be3, 0.0)
    f_b1 = _nontrivial(b1, 0.0)
    f_b2 = _nontrivial(b2, 0.0)
    f_b3 = _nontrivial(b3, 0.0)
    flags = (f_g1, f_g2, f_g3, f_b1, f_b2, f_b3)

    if flags not in _BUILD_CACHE:
        _BUILD_CACHE[flags] = _build(flags)
    nc = _BUILD_CACHE[flags]

    # stacked per-head weights [H, C, D] -> [C, H*D]; attention scale folded
    # into Wq
    wq_h = (np.transpose(np.asarray(Wq, np.float32), (1, 0, 2))
            .reshape(C, HD) * (D ** -0.5)).astype(bf)
    wk_h = np.transpose(np.asarray(Wk, np.float32), (1, 0, 2)).reshape(C, HD).astype(bf)
    wv_h = np.transpose(np.asarray(Wv, np.float32), (1, 0, 2)).reshape(C, HD).astype(bf)
    w1_h = np.asarray(W1, np.float32).astype(bf)
    w2_h = np.asarray(W2, np.float32).astype(bf)
    w3_h = np.asarray(W3, np.float32).astype(bf)

    in_maps = []
    for b in range(NCORES):
        xt = np.ascontiguousarray(x[b].T)
        yt = np.ascontiguousarray(y[b].T)
        m = {
            "xT": xt, "yT": yt,
            "xTb": xt.astype(bf), "yTb": yt.astype(bf),
            "wq": wq_h, "wk": wk_h, "wv": wv_h,
            "w1": w1_h, "w2": w2_h, "w3": w3_h,
        }
        if f_g1:
            m["g1"] = np.asarray(g1, np.float32).reshape(C, 1)
            m["be1"] = np.asarray(be1, np.float32).reshape(C, 1)
        if f_g2:
            m["g2"] = np.asarray(g2, np.float32).reshape(C, 1)
            m["be2"] = np.asarray(be2, np.float32).reshape(C, 1)
        if f_g3:
            m["g3"] = np.asarray(g3, np.float32).reshape(C, 1)
            m["be3"] = np.asarray(be3, np.float32).reshape(C, 1)
        if f_b1:
            m["b1"] = np.asarray(b1, np.float32).reshape(C, 1)
        if f_b2:
            m["b2"] = np.asarray(b2, np.float32).reshape(F, 1)
        if f_b3:
            m["b3"] = np.asarray(b3, np.float32).reshape(C, 1)
        in_maps.append(m)

    global _LAST_IN_MAPS
    _LAST_IN_MAPS = in_maps
    res = bass_utils.run_bass_kernel_spmd(nc, in_maps, core_ids=list(range(NCORES)))
    out = np.stack([np.ascontiguousarray(r["OT"].T) for r in res.results])
    return out.astype(np.float32)

